# revision 58
# baseline (speedup 1.0000x reference)
"""Trainium2 Bass kernel for nn_Attention_4415226380830 (XCA-style channel attention).

Reference computation (per batch b):
  qkv = conv1x1(x, qkv_w)                  # [3C, H, W]
  qkv = dwconv3x3(qkv, dw_w)               # depthwise, SAME zero pad
  q, k, v = split(qkv)                     # each [C, H, W]
  per head h (C=192, HEADS=4, DH=48):
    qh, kh L2-normalized over HW
    G = qh @ kh.T * temperature            # [DH, DH]
    attn = softmax(G, axis=-1)
    outh = attn @ vh                       # [DH, HW]
  out = conv1x1(concat(outh), proj_w)

Sharding: 8 cores = (batch b, spatial half) : each core owns 128 rows of one
batch image (+1 halo row each side for the depthwise conv).  The Gram matrix
and L2 norms are global over HW, so each core emits *partial* Gram/sumsq;
the (tiny, 192x192-per-batch) softmax is computed on host between two SPMD
launches.  v (dwconv'd) is spilled to DRAM in fp16 and re-read by phase 2.

Phase 1 (per core): 1x1 qkv conv (fp32r matmuls) -> depthwise 3x3 (fp16,
split across Vector/GpSimd/PE-diag engines) -> PE transposes of q,k ->
Gram partial (PSUM accumulation) + per-channel sumsq partial; v spilled.
Phase 2 (per core): out = attn @ v (block-diag matmul) -> proj 1x1 -> DRAM.
"""

import os
import numpy as np
from contextlib import ExitStack

import concourse.bass as bass
from concourse import bacc
import concourse.mybir as mybir
import concourse.tile as tile
from concourse.bass_utils import run_bass_kernel_spmd
from concourse.masks import make_identity

F32 = mybir.dt.float32
F32R = mybir.dt.float32r
F16 = mybir.dt.float16

# Problem constants (hardcoded; kernel.py must be self-contained)
B = 4
C = 192
HEADS = 4
DH = C // HEADS          # 48
H = 256
W = 256
C3 = 3 * C               # 576
N_CORES = 8
EPS = 1e-12

ROWS = H // 2            # rows per core (two cores per batch image)
MROWS = 16               # rows per macro-tile
NMACRO = ROWS // MROWS   # 8
WIN_ROWS = MROWS + 2     # qkv rows needed per macro (1 halo each side)
PXM = MROWS * W          # px per macro-tile output (4096)
SUB = 512                # px per conv psum substep (2 rows)
NSUB = (WIN_ROWS * W) // SUB   # conv substeps per macro (9)
NDWSUB = PXM // SUB      # dw-psum substeps per macro for PE chunks (8)

# channel chunks of the 576 qkv channels
CHUNKS = [(0, 128), (128, 256), (256, 384), (384, 512), (512, 576)]

# 3x3 taps: (dy, dx), dy/dx in {-1,0,1}; win row offset = 1+dy -> rows (t+dy+1)
TAPS = [(dy, dx) for dy in (-1, 0, 1) for dx in (-1, 0, 1)]
# per-chunk split of the 9 taps between the tensor engine (diagonal matmuls
# into PSUM, evacuated to initialize the accumulator) and the vector engine
# (scalar_tensor_tensor accumulate in SBUF).  Chunk 1's PE taps are exactly
# the dx==0 set (+1) so its vector taps never need the shifted copy.
# taps computed as Pool-engine multiply (broadcast weight) + DVE add --
# they come off the tensor engine, which is the phase-1 bottleneck.
POOL_TAPS = {0: [], 1: [], 2: [],
             3: [], 4: []}
PE_TAPS = {0: [], 1: [(-1, 0), (0, 0), (1, 0), (-1, -1)],
           2: TAPS,
           3: [t for t in TAPS if t not in POOL_TAPS[3]],
           4: [t for t in TAPS if t not in POOL_TAPS[4]]}
PE_CHUNKS = [ci for ci in range(5) if PE_TAPS[ci]]
# flat order of all pool taps for the w_rep constant tile
POOL_TAP_LIST = [(ci, t) for ci in range(5) for t in POOL_TAPS[ci]]
# vector-engine tap order: dx != 0 taps first (they read `win` directly);
# dx == 0 taps last so the Pool-engine shifted copy (win2) has time to land.
TAPS_V = [t for t in TAPS if t[1] != 0] + [t for t in TAPS if t[1] == 0]
# conv substeps per macro window (rows covered per PSUM fill): 4 rows =
# 1024 px per 2-bank psum tile, trailing 2-row step for the odd window.
CONV_STEPS = [(r, min(4, WIN_ROWS - r)) for r in range(0, WIN_ROWS, 4)]


def _build_phase1():
    nc = bacc.Bacc("TRN2", target_bir_lowering=False, debug=False,
                   num_devices=N_CORES)
    x_loc = nc.dram_tensor("x_loc", [C, ROWS + 2, W], F16, kind="ExternalInput").ap()
    qkv_wT = nc.dram_tensor("qkv_wT", [C, C3], F16, kind="ExternalInput").ap()
    dw_flat = nc.dram_tensor("dw_flat", [C3, 9], F32, kind="ExternalInput").ap()
    # diagonal dw-weight matrices for PE chunks: [n_pe, 9, 128, 128] fp16
    dw_diag = nc.dram_tensor("dw_diag", [len(PE_CHUNKS), 9, 128, 128], F16,
                             kind="ExternalInput").ap()
    # identity blocks: cols 0:128 = I128; cols 128:192 rows 64:128 = I64
    ident_in = nc.dram_tensor("ident_in", [128, 192], F16,
                              kind="ExternalInput").ap()

    gram_out = nc.dram_tensor("gram_part", [128, 384], F32, kind="ExternalOutput").ap()
    sumsq_out = nc.dram_tensor("sumsq_part", [128, 3 * NMACRO], F32,
                               kind="ExternalOutput").ap()
    v_out = nc.dram_tensor("v_sp", [C, ROWS * W], F16, kind="ExternalOutput").ap()

    with ExitStack() as ctx:
        tc = ctx.enter_context(tile.TileContext(nc))
        consts = ctx.enter_context(tc.tile_pool(name="consts", bufs=1))
        xpool = ctx.enter_context(tc.tile_pool(name="xpool", bufs=2))
        winp = ctx.enter_context(tc.tile_pool(name="winp", bufs=5))
        accp = ctx.enter_context(tc.tile_pool(name="accp", bufs=5))
        qkTp = ctx.enter_context(tc.tile_pool(name="qkTp", bufs=4))
        junkp = ctx.enter_context(tc.tile_pool(name="junkp", bufs=1))
        tmpp = ctx.enter_context(tc.tile_pool(name="tmpp", bufs=3))
        psum_conv = ctx.enter_context(tc.tile_pool(name="ps_conv", bufs=2, space="PSUM"))
        psum_dw = ctx.enter_context(tc.tile_pool(name="ps_dw", bufs=2, space="PSUM"))
        psum_t = ctx.enter_context(tc.tile_pool(name="ps_t", bufs=1, space="PSUM"))
        psum_g = ctx.enter_context(tc.tile_pool(name="ps_g", bufs=1, space="PSUM"))

        # ---- constants in SBUF ----
        wa = consts.tile([128, C3], F16, tag="wa")       # qkv_wT rows 0:128
        wb = consts.tile([64, C3], F16, tag="wb")        # rows 128:192
        nc.sync.dma_start(out=wa, in_=qkv_wT[0:128, :])
        nc.sync.dma_start(out=wb, in_=qkv_wT[128:192, :])
        dw_sb = consts.tile([128, len(CHUNKS), 9], F32, tag="dw")
        for ci, (c0, c1) in enumerate(CHUNKS):
            nc.sync.dma_start(out=dw_sb[0:c1 - c0, ci, :], in_=dw_flat[c0:c1, :])
        dwdiag_sb = consts.tile([128, len(PE_CHUNKS), 9, 128], F16, tag="dwdiag")
        nc.sync.dma_start(out=dwdiag_sb, in_=dw_diag.rearrange("a b p c -> p a b c"))
        ident = consts.tile([128, 192], F16, tag="ident")
        nc.sync.dma_start(out=ident, in_=ident_in)
        sumsq_sb = consts.tile([128, 3 * NMACRO], F32, tag="ssq")
        # broadcast pool-tap weights: w_rep[:, i, :] = dw weight replicated 256x
        w_rep = None
        if POOL_TAP_LIST:
            w_rep = consts.tile([128, len(POOL_TAP_LIST), W], F16, tag="wrep")
        for i, (ci, (dy, dx)) in enumerate(POOL_TAP_LIST):
            c0, c1 = CHUNKS[ci]
            t = (dy + 1) * 3 + (dx + 1)
            src = bass.AP(tensor=dw_sb.tensor, offset=dw_sb[0:c1 - c0, ci, t:t + 1].offset,
                          ap=[[dw_sb.ap[0][0], c1 - c0], [0, W]])
            nc.vector.tensor_copy(out=w_rep[0:c1 - c0, i, :], in_=src)

        gram_ps = psum_g.tile([128, 384], F32)   # persistent Gram accumulator

        for mj in range(NMACRO):
            # ---- load x rows [MROWS*mj, MROWS*mj + WIN_ROWS) of x_loc ----
            r0 = MROWS * mj
            xa = xpool.tile([128, WIN_ROWS, W], F16, tag="xa")
            xb = xpool.tile([64, WIN_ROWS, W], F16, tag="xb")
            nc.sync.dma_start(out=xa, in_=x_loc[0:128, r0:r0 + WIN_ROWS, :])
            nc.sync.dma_start(out=xb, in_=x_loc[128:192, r0:r0 + WIN_ROWS, :])
            xa_f = xa.rearrange("p r w -> p (r w)")
            xb_f = xb.rearrange("p r w -> p (r w)")

            accs = {}
            for ci in (0, 1, 2, 3, 4):
                c0, c1 = CHUNKS[ci]
                cp = c1 - c0
                # ---- 1x1 conv: win[cp, WIN_ROWS, 258] fp16 (zero side cols) ----
                win = winp.tile([128, WIN_ROWS, 258], F16, tag="win")
                # both pad columns (0 and 257) in one strided memset
                nc.gpsimd.memset(win[0:cp, :, 0:258:257], 0.0)
                for (sr, nr) in CONV_STEPS:
                    npx = nr * W
                    ps = psum_conv.tile([128, 2 * SUB], F32, tag="pc")
                    for h in range(npx // SUB):
                        o = sr * W + h * SUB
                        hs = slice(h * SUB, (h + 1) * SUB)
                        nc.tensor.matmul(ps[0:cp, hs], wa[:, c0:c1],
                                         xa_f[:, o:o + SUB],
                                         start=True, stop=False)
                        nc.tensor.matmul(ps[0:cp, hs], wb[:, c0:c1],
                                         xb_f[:, o:o + SUB],
                                         start=False, stop=True)
                    # evacuate (cast fp16) into window rows sr..sr+nr cols 1:257
                    dst = win[0:cp, sr:sr + nr, 1:257]
                    src = ps[0:cp, 0:npx].rearrange("p (r w) -> p r w", w=W)
                    nc.scalar.copy(out=dst, in_=src)

                # ---- depthwise 3x3 over MROWS output rows ----
                ptaps = PE_TAPS[ci]
                vtaps = [t for t in TAPS_V if t not in ptaps]
                acc = accp.tile([128, PXM], F16, tag="acc")
                acc3 = acc.rearrange("p (r w) -> p r w", w=W)
                if ptaps:
                    pi = PE_CHUNKS.index(ci)
                    for si in range(NDWSUB):
                        dps = psum_dw.tile([128, SUB], F32, tag="pd")
                        # substep covers output rows 2*si, 2*si+1
                        for ti, (dy, dx) in enumerate(ptaps):
                            t = (dy + 1) * 3 + (dx + 1)
                            src = win[0:cp, 2 * si + 1 + dy:2 * si + 3 + dy,
                                      1 + dx:257 + dx]
                            nc.tensor.matmul(
                                dps[0:cp, :], dwdiag_sb[0:cp, pi, t, 0:cp],
                                src, start=(ti == 0),
                                stop=(ti == len(ptaps) - 1))
                        nc.scalar.copy(out=acc[0:cp, si * SUB:(si + 1) * SUB],
                                       in_=dps[0:cp, :])
                pltaps = POOL_TAPS[ci]
                tmps = []
                for (dy, dx) in pltaps:
                    i = POOL_TAP_LIST.index((ci, (dy, dx)))
                    tmp = tmpp.tile([128, PXM], F16, tag="tmp")
                    wbc = bass.AP(tensor=w_rep.tensor,
                                  offset=w_rep[0:cp, i, :].offset,
                                  ap=[[w_rep.ap[0][0], cp], [0, MROWS], [1, W]])
                    nc.gpsimd.tensor_tensor(
                        out=tmp.rearrange("p (r w) -> p r w", w=W)[0:cp],
                        in0=win[0:cp, 1 + dy:1 + dy + MROWS, 1 + dx:257 + dx],
                        in1=wbc, op=mybir.AluOpType.mult)
                    tmps.append(tmp)
                if vtaps:
                    # scalar_tensor_tensor has no 2x DVE uop, so alignment
                    # is irrelevant — read the window directly for all taps.
                    first = not ptaps
                    for (dy, dx) in sorted(vtaps):
                        t = (dy + 1) * 3 + (dx + 1)
                        sc = dw_sb[0:cp, ci, t:t + 1]
                        src = win[0:cp, 1 + dy:1 + dy + MROWS,
                                  1 + dx:257 + dx]
                        if first:
                            nc.vector.tensor_scalar_mul(acc3[0:cp], src, sc)
                            first = False
                        else:
                            nc.vector.scalar_tensor_tensor(
                                out=acc3[0:cp], in0=src, scalar=sc,
                                in1=acc3[0:cp], op0=mybir.AluOpType.mult,
                                op1=mybir.AluOpType.add)
                for tmp in tmps:
                    nc.vector.tensor_tensor(out=acc[0:cp, :], in0=acc[0:cp, :],
                                            in1=tmp[0:cp, :],
                                            op=mybir.AluOpType.add)
                accs[ci] = acc

                # ---- v chunks: spill to DRAM ----
                if ci == 3:
                    nc.sync.dma_start(
                        out=v_out[0:128, mj * PXM:(mj + 1) * PXM],
                        in_=accs[ci][0:128, :])
                elif ci == 4:
                    nc.sync.dma_start(
                        out=v_out[128:192, mj * PXM:(mj + 1) * PXM],
                        in_=accs[ci][0:64, :])

            # ---- sumsq partials for q,k chunks (0,1,2) on ScalarE ----
            for ci in range(3):
                cp = CHUNKS[ci][1] - CHUNKS[ci][0]
                junk = junkp.tile([128, PXM], F16, tag="junk")
                nc.scalar.activation(
                    out=junk[0:cp, :], in_=accs[ci][0:cp, :],
                    func=mybir.ActivationFunctionType.Square,
                    accum_out=sumsq_sb[0:cp, ci * NMACRO + mj:ci * NMACRO + mj + 1])

            # ---- transposes of q,k + Gram accumulation (2 n-chunks/bank) ----
            aq0, aqk, ak1 = accs[0], accs[1], accs[2]
            for pr in range(PXM // 256):
                tps = psum_t.tile([128, 768], F16, tag="pt")
                for h in range(2):
                    cs = slice(pr * 256 + h * 128, pr * 256 + h * 128 + 128)
                    o = 384 * h
                    # one PSUM bank shared by 6 transposes: only the first
                    # uses start=True (start zeroes the whole 2KB bank
                    # lazily); the rest overwrite their still-pending bytes.
                    nc.tensor.matmul(tps[:, o:o + 128], aq0[:, cs],
                                     ident[:, 0:128], is_transpose=True,
                                     start=(h == 0), stop=False,
                                     skip_group_check=True)
                    # acc1's full transpose yields (q-ch 128:192 | k-ch 0:64)
                    # columns directly — no partial-partition transposes
                    # (those hard-fault the device).
                    nc.tensor.matmul(tps[:, o + 128:o + 256], aqk[:, cs],
                                     ident[:, 0:128], is_transpose=True,
                                     start=False, stop=False,
                                     skip_group_check=True)
                    nc.tensor.matmul(tps[:, o + 256:o + 384], ak1[:, cs],
                                     ident[:, 0:128], is_transpose=True,
                                     start=False, stop=(h == 1),
                                     skip_group_check=True)
                qkT = qkTp.tile([128, 768], F16, tag="qkT")
                nc.scalar.copy(out=qkT, in_=tps)
                for h in range(2):
                    o = 384 * h
                    first_g = (mj == 0 and pr == 0 and h == 0)
                    last_g = (mj == NMACRO - 1 and pr == PXM // 256 - 1
                              and h == 1)
                    # Gram bank: only the very first matmul may use
                    # start=True; the second region's first write lands on
                    # still-pending bytes and overwrites.
                    nc.tensor.matmul(gram_ps[0:128, 0:192],
                                     qkT[:, o:o + 128],
                                     qkT[:, o + 192:o + 384],
                                     start=first_g, stop=last_g,
                                     skip_group_check=True)
                    nc.tensor.matmul(gram_ps[0:64, 192:384],
                                     qkT[:, o + 128:o + 192],
                                     qkT[:, o + 192:o + 384],
                                     start=False, stop=last_g,
                                     skip_group_check=True)

        # ---- final outputs ----
        gram_sb = consts.tile([128, 384], F32, tag="gsb")
        nc.vector.memset(gram_sb[64:128, 192:384], 0.0)
        nc.vector.tensor_copy(out=gram_sb[:, 0:192], in_=gram_ps[0:128, 0:192])
        nc.vector.tensor_copy(out=gram_sb[0:64, 192:384],
                              in_=gram_ps[0:64, 192:384])
        nc.sync.dma_start(out=gram_out, in_=gram_sb)
        nc.sync.dma_start(out=sumsq_out, in_=sumsq_sb)
    nc.compile()
    return nc


def _build_phase2():
    nc = bacc.Bacc("TRN2", target_bir_lowering=False, debug=False,
                   num_devices=N_CORES)
    v_in = nc.dram_tensor("v_sp", [C, ROWS * W], F16, kind="ExternalInput").ap()
    # mwT = (proj_w @ block_diag(attn_heads)).T, folded on host: out = mwT.T @ v
    mwT = nc.dram_tensor("mwT", [C, C], F16, kind="ExternalInput").ap()
    out_loc = nc.dram_tensor("out_loc", [C, ROWS * W], F32, kind="ExternalOutput").ap()

    BT = 4096                # px per DMA tile (8 x 512 compute substeps)
    NT = ROWS * W // BT      # 32 tiles
    with ExitStack() as ctx:
        tc = ctx.enter_context(tile.TileContext(nc))
        consts = ctx.enter_context(tc.tile_pool(name="consts", bufs=1))
        vpool = ctx.enter_context(tc.tile_pool(name="vpool", bufs=4))
        aopool = ctx.enter_context(tc.tile_pool(name="aopool", bufs=4))
        ps_pj = ctx.enter_context(tc.tile_pool(name="ps_pj", bufs=3, space="PSUM"))

        mw = consts.tile([96, 2, C], F16, tag="mw")   # mwT rows (0:96, 96:192)
        nc.sync.dma_start(out=mw[:, 0, :], in_=mwT[0:96, :])
        nc.sync.dma_start(out=mw[:, 1, :], in_=mwT[96:192, :])

        for t in range(NT):
            px = slice(t * BT, (t + 1) * BT)
            va = vpool.tile([96, BT], F16, tag="va")
            vb = vpool.tile([96, BT], F16, tag="vb")
            nc.sync.dma_start(out=va, in_=v_in[0:96, px])
            nc.sync.dma_start(out=vb, in_=v_in[96:192, px])
            oja = aopool.tile([128, BT], F32, tag="oja")
            ojb = aopool.tile([64, BT], F32, tag="ojb")
            for h in range(BT // SUB):
                hs = slice(h * SUB, (h + 1) * SUB)
                pja = ps_pj.tile([128, SUB], F32, tag="pja")
                pjb = ps_pj.tile([64, SUB], F32, tag="pjb")
                nc.tensor.matmul(pja, mw[:, 0, 0:128], va[:, hs],
                                 start=True, stop=False)
                nc.tensor.matmul(pja, mw[:, 1, 0:128], vb[:, hs],
                                 start=False, stop=True)
                nc.tensor.matmul(pjb, mw[:, 0, 128:192], va[:, hs],
                                 start=True, stop=False)
                nc.tensor.matmul(pjb, mw[:, 1, 128:192], vb[:, hs],
                                 start=False, stop=True)
                nc.scalar.copy(out=oja[:, hs], in_=pja)
                nc.vector.tensor_copy(out=ojb[:, hs], in_=pjb)
            nc.sync.dma_start(out=out_loc[0:128, px], in_=oja)
            nc.sync.dma_start(out=out_loc[128:192, px], in_=ojb)
    nc.compile()
    return nc


_NC1 = None
_NC2 = None
_LAST_R1 = None
_LAST_R2 = None


def _get_programs():
    global _NC1, _NC2
    if _NC1 is None:
        _NC1 = _build_phase1()
        _NC2 = _build_phase2()
    return _NC1, _NC2


def kernel(x, qkv_w, dw_w, proj_w, temperature, _trace=False):
    x = np.asarray(x, dtype=np.float32)
    qkv_w = np.asarray(qkv_w, dtype=np.float32)
    dw_w = np.asarray(dw_w, dtype=np.float32)
    proj_w = np.asarray(proj_w, dtype=np.float32)
    temperature = np.asarray(temperature, dtype=np.float32)

    nc1, nc2 = _get_programs()

    qkv_wT = np.ascontiguousarray(qkv_w[:, :, 0, 0].T).astype(np.float16)
    dw_flat = np.ascontiguousarray(dw_w[:, 0].reshape(C3, 9))
    dw_diag = np.zeros((len(PE_CHUNKS), 9, 128, 128), np.float16)
    for pi, ci in enumerate(PE_CHUNKS):
        c0, c1 = CHUNKS[ci]
        for t in range(9):
            d = np.zeros(128, np.float32)
            d[0:c1 - c0] = dw_flat[c0:c1, t]
            dw_diag[pi, t] = np.diag(d).astype(np.float16)

    ident_np = np.zeros((128, 192), np.float16)
    ident_np[:, 0:128] = np.eye(128)
    ident_np[64:128, 128:192] = np.eye(64)

    in_maps1 = []
    for core in range(N_CORES):
        b, half = divmod(core, 2)
        base = half * ROWS
        x_pad = np.zeros((C, ROWS + 2, W), np.float16)
        lo, hi = base - 1, base + ROWS + 1
        slo, shi = max(lo, 0), min(hi, H)
        x_pad[:, slo - lo:shi - lo, :] = x[b, :, slo:shi, :].astype(np.float16)
        in_maps1.append({"x_loc": x_pad, "qkv_wT": qkv_wT,
                         "dw_flat": dw_flat, "dw_diag": dw_diag,
                         "ident_in": ident_np})

    global _LAST_R1, _LAST_R2
    r1 = run_bass_kernel_spmd(nc1, in_maps1, core_ids=list(range(N_CORES)),
                              trace=_trace)
    _LAST_R1 = r1

    # ---- host: combine partials, softmax, fold proj into attn (tiny) ----
    proj_w2 = proj_w[:, :, 0, 0].astype(np.float64)
    mwTs = np.zeros((B, C, C), np.float16)
    for b in range(B):
        ra, rb = r1.results[2 * b], r1.results[2 * b + 1]
        gp = ra["gram_part"].astype(np.float64) + rb["gram_part"].astype(np.float64)
        G = np.concatenate([gp[:, 0:192], gp[0:64, 192:384]], axis=0)  # [192,192] q x k
        ss = ra["sumsq_part"].astype(np.float64) + rb["sumsq_part"].astype(np.float64)
        ss = ss.reshape(128, 3, NMACRO).sum(axis=2)     # [128, 3]
        ssf = ss.T.reshape(-1)                          # ch 0..384 (q then k)
        qn = np.maximum(np.sqrt(ssf[0:192]), EPS)
        kn = np.maximum(np.sqrt(ssf[192:384]), EPS)
        attn_bd = np.zeros((C, C))
        for h in range(HEADS):
            sl = slice(h * DH, (h + 1) * DH)
            Gh = G[sl, sl] / np.outer(qn[sl], kn[sl]) * float(temperature[h, 0, 0])
            Gh = Gh - Gh.max(axis=1, keepdims=True)
            e = np.exp(Gh)
            attn_bd[sl, sl] = e / e.sum(axis=1, keepdims=True)
        # out = proj_w @ attn_bd @ v  ->  fold: mwT = (proj_w @ attn_bd).T
        mwTs[b] = (proj_w2 @ attn_bd).T.astype(np.float16)

    in_maps2 = []
    for core in range(N_CORES):
        b = core // 2
        in_maps2.append({"v_sp": r1.results[core]["v_sp"], "mwT": mwTs[b]})
    r2 = run_bass_kernel_spmd(nc2, in_maps2, core_ids=list(range(N_CORES)),
                              trace=_trace)
    _LAST_R2 = r2

    out = np.zeros((B, C, H, W), np.float32)
    for core in range(N_CORES):
        b, half = divmod(core, 2)
        out[b, :, half * ROWS:(half + 1) * ROWS, :] = \
            r2.results[core]["out_loc"].reshape(C, ROWS, W)
    return out
